# revision 2
# baseline (speedup 1.0000x reference)
"""Trainium2 Bass kernel for DCKModule (involution), v3: transposed layout.

Math per image (fp32 reference):
  x  = relu(W1 @ guide * bn_scale + bn_bias)        # (64, 9216)
  df = W2 @ x                                       # (784, 9216)
  out[g,gc,p] = sum_k df[g,k,p] * fpad[g,gc, p+off_k] + feature[g,gc,p]

Mapping: data-parallel over batch (1 image / NeuronCore). Inside a core the
layout is TRANSPOSED: partitions = image column j (96 of 128), free = (c, r).
The 7 column shifts dj cannot be partition offsets (engine SBUF access must
start at partition 0/32/64/96), so the host pre-shifts the padded feature map
into 7 DRAM copies fsh[dj][j, c, r'] = fpad[c, r', j+dj]; all tap shifts are
then free-dim offsets. Channels run in 4 slices of 64 so the 7 active slabs
fit SBUF; slabs for the next slice refill as each dj tap-group retires.

Why this layout wins: df stays COMPACT (784 x 9216, not the 16x gc-broadcast
of the c-partition layout), so the PSUM->SBUF bf16 cast is tiny and the
gc-broadcast is free via a stride-0 access-pattern dim in the DVE multiply.
All elementwise work is bf16 (DVE 2x mode; tolerance 2e-2).

Engines: per (slice, super-block of 24 rows) the 49 tap-products are
multiplied on DVE (40 taps, one [96,4,16,24] tensor_tensor each) and Pool
(9 taps, per-group 3D ops); every product is accumulated on the TensorEngine
into fp32 PSUM by identity-matmul accumulation (3 banks of 512, start= the
residual init read from the dj=3 slab). ACT casts df (PSUM->SBUF bf16) and
drains PSUM transposing (c,r)->(r,c) so the output DMA runs 256B-contiguous.
"""

import numpy as np
import ml_dtypes

import concourse.bass as bass
import concourse.mybir as mybir
import concourse.tile as tile
from concourse import bacc, bass_utils

B, C, H, W = 8, 256, 96, 96
K7, PAD, G, GC, R = 7, 3, 16, 16, 64
HP = H + 2 * PAD          # 102
PIX = H * W               # 9216
BN_EPS = 1e-5

NSL = 4                   # channel slices
GSL = G // NSL            # groups per slice = 4
CSL = C // NSL            # channels per slice = 64
ABLK = 8                  # rows per PSUM acc bank (64ch*8 = 512 fp32 = 1 bank)
NAB = 3                   # acc banks per super-block
SBLK = ABLK * NAB         # 24 rows per super-block
NSB = H // SBLK           # 4 super-blocks per slice
FREE = GSL * GC * SBLK    # 1536 elems per DVE mult
NK = K7 * K7              # 49

F32 = mybir.dt.float32
BF16 = mybir.dt.bfloat16
BF = ml_dtypes.bfloat16
TRACE = False

_CACHE = {}

# taps t = dj*7 + di; these go to Pool (others to DVE)
POOL_MULT = frozenset({6, 12, 17, 23, 28, 34, 39, 45, 48})
DF_EMIT_AT = (8, 16, 24, 32)     # taps after which next-sb df row-groups emit
PROD_BUFS = 8                    # DVE prod rotation depth
PRODP_BUFS = 6                   # Pool prod rotation depth (Pool runs ahead)


def _build_nc():
    mult = mybir.AluOpType.mult
    nc = bacc.Bacc(None, target_bir_lowering=False)
    gm_d = nc.dram_tensor("gm", [C, PIX], BF16, kind="ExternalInput")
    w1_d = nc.dram_tensor("w1pt", [C, R], BF16, kind="ExternalInput")
    bias_d = nc.dram_tensor("bias", [R, 1], F32, kind="ExternalInput")
    w2_d = nc.dram_tensor("w2ct", [R, NK * G], BF16, kind="ExternalInput")
    fsh_d = nc.dram_tensor("fsh", [K7 * W, C * HP], BF16, kind="ExternalInput")
    id_d = nc.dram_tensor("idm", [W, W], BF16, kind="ExternalInput")
    out_d = nc.dram_tensor("out", [W, H * C], F32, kind="ExternalOutput")

    with tile.TileContext(nc) as tc:
        with tc.tile_pool(name="persist", bufs=1) as persist, \
             tc.tile_pool(name="slabs", bufs=1) as slabs:

            w1_sb = persist.tile([128, 2 * R], BF16, tag="w1", name="w1sb")
            bias_sb = persist.tile([R, 1], F32, tag="bias", name="biassb")
            w2_sb = persist.tile([R, NK * G], BF16, tag="w2", name="w2sb")
            ident = persist.tile([W, W], BF16, tag="idm", name="identsb")
            x_sb = persist.tile([R, PIX], BF16, tag="x", name="xsb")

            for ck in range(2):
                nc.sync.dma_start(out=w1_sb[:, ck * R:(ck + 1) * R],
                                  in_=w1_d[ck * 128:(ck + 1) * 128, :])
            nc.sync.dma_start(out=bias_sb[:], in_=bias_d[:])
            nc.sync.dma_start(out=w2_sb[:], in_=w2_d[:])
            nc.sync.dma_start(out=ident[:], in_=id_d[:])

            def load_slab(dj, cs):
                sl = slabs.tile([W, CSL * HP], BF16, tag=f"slab{dj}",
                                name=f"slab{dj}")
                nc.sync.dma_start(
                    out=sl[:],
                    in_=fsh_d[dj * W:(dj + 1) * W,
                              cs * CSL * HP:(cs + 1) * CSL * HP])
                return sl

            dfpool = tc.alloc_tile_pool(name="dfpool", bufs=2)
            psdf = tc.alloc_tile_pool(name="psdf", bufs=2, space="PSUM")

            def df_rowgroup(cs, df, r0, rg):
                """Two rows of compact df -> psum -> bf16 cast into df."""
                dp = psdf.tile([W, 2 * GSL * NK], F32,
                               tag="dfps", name="dfps")
                for rr in range(2):
                    r = r0 + rg * 2 + rr
                    nc.tensor.matmul(
                        dp[:, rr * GSL * NK:(rr + 1) * GSL * NK],
                        x_sb[:, r * W:(r + 1) * W],
                        w2_sb[:, cs * GSL * NK:(cs + 1) * GSL * NK],
                        start=True, stop=True)
                cin = dp[:].rearrange("p (rr g k) -> p rr g k",
                                      rr=2, g=GSL)
                dst = df[:].rearrange(
                    "p (g k r) -> p g k r", g=GSL, k=NK)[
                        :, :, :, rg * 2:(rg + 1) * 2].transpose(
                        [0, 3, 1, 2])
                nc.scalar.copy(dst, cin)

            def new_df():
                return dfpool.tile([W, GSL * NK * SBLK], BF16,
                                   tag="df", name="df")

            df_cur = new_df()
            rg0 = 0

            # ---- phase 1: x = relu(W1p^T @ guide + bias), bf16 ----
            with tc.tile_pool(name="gpool", bufs=1) as gpool, \
                 tc.tile_pool(name="psx", bufs=2, space="PSUM") as psx:
                g_sb = [gpool.tile([128, PIX], BF16, tag=f"g{ck}",
                                   name=f"gsb{ck}") for ck in range(2)]
                for q in range(4):
                    for ck in range(2):
                        nc.sync.dma_start(
                            out=g_sb[ck][:, q * (PIX // 4):(q + 1) * (PIX // 4)],
                            in_=gm_d[ck * 128:(ck + 1) * 128,
                                     q * (PIX // 4):(q + 1) * (PIX // 4)])
                slab = [load_slab(dj, 0) for dj in range(K7)]
                NCH = 512
                for ch in range(PIX // NCH):
                    xp = psx.tile([R, NCH], F32, tag="xps", name="xps")
                    for ck in range(2):
                        nc.tensor.matmul(
                            xp[:], w1_sb[:, ck * R:(ck + 1) * R],
                            g_sb[ck][:, ch * NCH:(ch + 1) * NCH],
                            start=(ck == 0), stop=(ck == 1))
                    nc.scalar.activation(
                        x_sb[:, ch * NCH:(ch + 1) * NCH], xp[:],
                        mybir.ActivationFunctionType.Relu,
                        bias=bias_sb[:], scale=1.0)
                    # emit df(0,0) row-groups as soon as x rows are covered
                    covered = (NCH * (ch + 1)) // W
                    while rg0 < SBLK // 2 and 2 * (rg0 + 1) <= covered:
                        df_rowgroup(0, df_cur, 0, rg0)
                        rg0 += 1

            # ---- phase 2: involution per (slice, super-block) ----
            with tc.tile_pool(name="prodpool", bufs=PROD_BUFS) as prodpool, \
                 tc.tile_pool(name="prodp", bufs=PRODP_BUFS) as prodp, \
                 tc.tile_pool(name="outpool", bufs=2) as outpool, \
                 tc.tile_pool(name="psacc", bufs=2, space="PSUM") as psacc:

                steps = [(cs, sb) for cs in range(NSL) for sb in range(NSB)]

                for si, (cs, sb) in enumerate(steps):
                    r0 = sb * SBLK
                    nxt = steps[si + 1] if si + 1 < len(steps) else None
                    df = df_cur
                    df_next = new_df() if nxt is not None else None
                    rg_next = 0

                    dfv = df[:].rearrange("p (g k r) -> p g k r",
                                          g=GSL, k=NK)
                    acc = [psacc.tile([W, GSL * GC * ABLK], F32,
                                      tag=f"acc{rb}", name=f"acc{rb}")
                           for rb in range(NAB)]
                    sv3 = slab[3][:].rearrange("p (c r) -> p c r", c=CSL)
                    for rb in range(NAB):
                        nc.tensor.matmul(
                            acc[rb][:], ident[:],
                            sv3[:, :, r0 + rb * ABLK + PAD:
                                r0 + rb * ABLK + PAD + ABLK],
                            start=True, stop=False)

                    for t in range(NK):
                        dj, di = divmod(t, K7)
                        sv = slab[dj][:].rearrange(
                            "p (g gc rp) -> p g gc rp", g=GSL, gc=GC)
                        if t in POOL_MULT:
                            prod = prodp.tile([W, FREE], BF16,
                                              tag="prodp", name="prodp")
                            for g in range(GSL):
                                nc.gpsimd.tensor_tensor(
                                    prod[:].rearrange(
                                        "p (g gc r) -> p g gc r",
                                        g=GSL, gc=GC)[:, g, :, :],
                                    dfv[:, g, t:t + 1, :].broadcast_to(
                                        [W, GC, SBLK]),
                                    sv[:, g, :, r0 + di:r0 + di + SBLK],
                                    mult)
                        else:
                            prod = prodpool.tile([W, FREE], BF16,
                                                 tag="prod", name="prod")
                            nc.vector.tensor_tensor(
                                prod[:].rearrange(
                                    "p (g gc r) -> p g gc r", g=GSL, gc=GC),
                                dfv[:, :, t:t + 1, :].broadcast_to(
                                    [W, GSL, GC, SBLK]),
                                sv[:, :, :, r0 + di:r0 + di + SBLK],
                                mult)
                        pvv = prod[:].rearrange(
                            "p (g gc r) -> p g gc r", g=GSL, gc=GC)
                        for rb in range(NAB):
                            nc.tensor.matmul(
                                acc[rb][:], ident[:],
                                pvv[:, :, :, rb * ABLK:(rb + 1) * ABLK],
                                start=False, stop=(t == NK - 1),
                                skip_group_check=True)
                        # emit next super-block's df in chunks mid-taps
                        if df_next is not None and t in DF_EMIT_AT:
                            for _ in range(SBLK // 2 // len(DF_EMIT_AT)):
                                df_rowgroup(nxt[0], df_next, nxt[1] * SBLK,
                                            rg_next)
                                rg_next += 1
                        # refill slab dj for the next slice once retired
                        if sb == NSB - 1 and di == K7 - 1 and cs + 1 < NSL:
                            slab[dj] = load_slab(dj, cs + 1)

                    # drain: transpose (c, r)->(r, c); DMA runs 256B each
                    osb = outpool.tile([W, GSL * GC * SBLK], F32,
                                       tag="osb", name="osb")
                    ov = osb[:].rearrange("p (r c) -> p r c", r=SBLK)
                    for rb in range(NAB):
                        nc.scalar.copy(
                            ov[:, rb * ABLK:(rb + 1) * ABLK, :].transpose(
                                [0, 2, 1]),
                            acc[rb][:].rearrange("p (c r) -> p c r", c=CSL))
                    nc.sync.dma_start(
                        out=out_d[:, :].rearrange("p (r c) -> p r c", r=H)[
                            :, r0:r0 + SBLK, cs * CSL:(cs + 1) * CSL],
                        in_=ov)

                    df_cur = df_next

            psdf.release()
            dfpool.release()

    if not nc.is_finalized():
        nc.finalize()
    return nc


def _host_prep(feature_map, guide_map, W1, bn_gamma, bn_beta, bn_mean,
               bn_var, W2):
    fm = np.asarray(feature_map, np.float32).reshape(B, C, H, W)
    gm = np.asarray(guide_map, np.float32).reshape(B, C, PIX).astype(BF)

    inv = bn_gamma / np.sqrt(bn_var + BN_EPS)
    W1p = (np.asarray(W1, np.float32) * inv[:, None])
    w1pt = np.ascontiguousarray(W1p.T).astype(BF)               # (256, 64)
    bias = (bn_beta - bn_mean * inv).astype(np.float32).reshape(R, 1)
    # W2 rows are (g, di*7+dj); reorder to columns (g, dj*7+di)
    W2r = np.asarray(W2, np.float32).reshape(G, K7, K7, R)       # [g, di, dj, o]
    w2ct = np.ascontiguousarray(
        W2r.transpose(3, 0, 2, 1).reshape(R, G * NK)).astype(BF)

    fpad = np.pad(fm, ((0, 0), (0, 0), (PAD, PAD), (PAD, PAD)))  # [B, C, HP, HP]
    ftr = fpad.transpose(0, 3, 1, 2).astype(BF)                  # [B, jpad, C, rpad]
    fsh = np.empty((B, K7, W, C * HP), BF)
    for dj in range(K7):
        fsh[:, dj] = ftr[:, dj:dj + W].reshape(B, W, C * HP)
    fsh = fsh.reshape(B, K7 * W, C * HP)

    idm = np.eye(W, dtype=BF)
    return fm, gm, w1pt, bias, w2ct, fsh, idm


def kernel(feature_map, guide_map, W1, bn_gamma, bn_beta, bn_mean, bn_var, W2):
    fm, gm, w1pt, bias, w2ct, fsh, idm = _host_prep(
        feature_map, guide_map, W1, bn_gamma, bn_beta, bn_mean, bn_var, W2)

    if "nc" not in _CACHE:
        _CACHE["nc"] = _build_nc()
    nc = _CACHE["nc"]

    in_maps = [dict(gm=gm[i], w1pt=w1pt, bias=bias, w2ct=w2ct,
                    fsh=fsh[i], idm=idm) for i in range(B)]
    _CACHE["in_maps"] = in_maps
    res = bass_utils.run_bass_kernel_spmd(
        nc, in_maps, core_ids=list(range(B)), trace=TRACE)
    _CACHE["last"] = res
    # out_d[j, r, c] -> [c, r, j]
    out = np.stack([np.ascontiguousarray(
        r["out"].reshape(W, H, C).transpose(2, 1, 0)) for r in res.results],
        axis=0)
    return out.reshape(B, C, H, W)
